# revision 23
# baseline (speedup 1.0000x reference)
"""nn_CAM_Module kernel for 8 Trainium2 NeuronCores (Bass/Tile).

Contract: kernel(**inputs) takes the FULL inputs (x: [16, 512, 64, 64] fp32,
gamma: [1] fp32) and returns the FULL output, sharding batch B=16 across the
8 cores (2 samples per core, gamma replicated) — per the data-parallel
sharding: every op is a per-sample bmm, no cross-core communication.

Per-sample computation (C=512 channels, N=H*W=4096):
  energy = xf @ xf.T                          (C,C), contraction over N on PE
  m_i    = min_j energy[i,j]                  (softmax(max-e) == softmax(m-e))
  P_ij   = exp(m_i - energy_ij), S_i = sum_j  (ACT, fused row-sum)
  out    = diag(1/S) @ (P @ xf)               (PE; P^T tiles via PE transpose)
  y      = gamma * out + x                    (epilogue split DVE / ACT+gps)

v5 layout: x is resident as bf16 (xf16, epilogue source) + fp8 (xfc, matmul
moving operand); the fp32 DMA landing zone (stage) is transient.  This
halves SBUF residency vs keeping fp32, so BOTH samples' chunks fit and the
read stream never throttles on pool recycling.  Loads are 2MB blocks (4KB
per partition line) for lower DMA descriptor overhead.

Engine balance per 512-col unit (measured ns): DVE copy ~420/1.2k,
ACT ~700/1.1k, gpsimd add ~1.1k (0.42 eff), DVE STT epilogue ~745.
Casts/copies are spread DVE/ACT/gpsimd so no engine exceeds the phase
cadence (DMA 2.6us/chunk in the energy phase, PE ~5.4us/pair in the
mm2+digest phase).
"""

import os
from contextlib import ExitStack

import numpy as np

B, C, H, W = 16, 512, 64, 64
N = H * W
N_CORES = 8
BPC = B // N_CORES
P = 128

MM_DT_NAME = os.environ.get("CAM_MM_DT", "fp8")

LAST_EXEC_TIME_NS = None
LAST_TRACE = None
LAST_PROFILE_JSON = None
_CACHE = {}


def _build(mm_dt_name):
    import concourse.mybir as mybir
    import concourse.tile as tile
    from concourse import bacc
    from concourse.masks import make_identity

    F32 = mybir.dt.float32
    BF16 = mybir.dt.bfloat16
    mm_dt = {
        "bf16": mybir.dt.bfloat16,
        "fp8": mybir.dt.float8e4,
        "f32": F32,
    }[mm_dt_name]
    DR = mm_dt in (mybir.dt.float8e4, mybir.dt.float8e5)

    CB = C // P          # 4 channel blocks
    KB = N // P          # 32 spatial chunks
    NCH_SZ = 512
    NCH = N // NCH_SZ    # 8 compute chunks
    LD_SZ = 1024
    NLD = N // LD_SZ     # 4 load blocks (2 chunks each)

    nc = bacc.Bacc(None, target_bir_lowering=False, debug=False)
    x = nc.dram_tensor("x", [BPC, C, N], F32, kind="ExternalInput")
    gamma = nc.dram_tensor("gamma", [1], F32, kind="ExternalInput")
    y = nc.dram_tensor("y", [BPC, C, N], F32, kind="ExternalOutput")

    with ExitStack() as ctx:
        tc = ctx.enter_context(tile.TileContext(nc))
        singles = ctx.enter_context(tc.tile_pool(name="singles", bufs=1))
        stage_pool = ctx.enter_context(tc.tile_pool(name="stage", bufs=3))
        xf16_pool = ctx.enter_context(tc.tile_pool(name="xf16", bufs=2 * NCH))
        xfc_pool = ctx.enter_context(tc.tile_pool(name="xfc", bufs=2 * NCH))
        xfT_pool = ctx.enter_context(tc.tile_pool(name="xfT", bufs=1))
        pmat_pool = ctx.enter_context(tc.tile_pool(name="pmat", bufs=2))
        pt_pool = ctx.enter_context(tc.tile_pool(name="pt", bufs=2))
        small = ctx.enter_context(tc.tile_pool(name="small", bufs=16))
        ytb_pool = ctx.enter_context(tc.tile_pool(name="ytb", bufs=2))
        yts_pool = ctx.enter_context(tc.tile_pool(name="yts", bufs=6))
        eps_pool = ctx.enter_context(tc.tile_pool(name="eps", bufs=4, space="PSUM"))
        tps_pool = ctx.enter_context(tc.tile_pool(name="tps", bufs=2, space="PSUM"))
        ops_pool = ctx.enter_context(tc.tile_pool(name="ops", bufs=2, space="PSUM"))

        states = {}

        def load_block(b, blk):
            """Issue one 2MB load block (sync HWDGE). Emission-hoistable:
            adds no compute work, just keeps the read stream fed."""
            st = states.setdefault(b, {"stage": [], "xf16": [], "xfc": []})
            if len(st["stage"]) > blk:
                return
            xv = x[b].rearrange("(cb p) n -> p cb n", p=P)
            nsl = slice(blk * LD_SZ, (blk + 1) * LD_SZ)
            stg = stage_pool.tile([P, CB, LD_SZ], F32, tag="stage",
                                  name=f"stg{b}_{blk}")
            if b == 0 and blk == 0:
                # split the very first load per-cb so the first cast (and
                # with it the first PE transpose) starts earlier
                for cb in range(CB):
                    nc.sync.dma_start(stg[:, cb, :], xv[:, cb, nsl])
            else:
                nc.sync.dma_start(stg[:], xv[:, :, nsl])
            st["stage"].append(stg)

        def copy_on(eng, out, in_):
            if eng == "v":
                nc.vector.tensor_copy(out=out, in_=in_)
            elif eng == "s":
                nc.scalar.copy(out=out, in_=in_)
            else:
                nc.gpsimd.tensor_copy(out=out, in_=in_)

        def cast16_chunk(b, ch, eng=None):
            """fp32 stage -> bf16 xf16 (epilogue source, fp8-cast source)."""
            st = states[b]
            if len(st["xf16"]) > ch:
                return
            load_block(b, ch // 2)
            blk, half = divmod(ch, 2)
            src = st["stage"][blk]
            xf16 = xf16_pool.tile([P, CB, NCH_SZ], BF16, tag="xf16",
                                  name=f"xf16_{b}_{ch}")
            csl = slice(half * NCH_SZ, (half + 1) * NCH_SZ)
            for cb in range(CB):
                copy_on(eng or ("v" if cb % 2 == 0 else "s"),
                        xf16[:, cb, :], src[:, cb, csl])
            st["xf16"].append(xf16)

        def cast8_chunk(b, ch, eng=None):
            """bf16 xf16 -> fp8 xfc (matmul moving operand + transpose src).
            16-bit source reads at 2x DVE rate; mostly on gpsimd, which is
            otherwise idle during the energy phases."""
            st = states[b]
            if len(st["xfc"]) > ch:
                return
            xf16 = st["xf16"][ch]
            xfc = xfc_pool.tile([P, CB, NCH_SZ], mm_dt, tag="xfc",
                                name=f"xfc{b}_{ch}")
            for cb in range(CB):
                copy_on(eng or ("v" if cb % 4 == 0 else "g"),
                        xfc[:, cb, :], xf16[:, cb, :])
            st["xfc"].append(xfc)

        # fp8 PE-transpose writes PSUM with element step 2 (16-bit write
        # packing): stage into a 2x-strided PSUM view, copy back strided.
        TW = 2 if DR else 1
        KPC = NCH_SZ // P  # transposes-k per n-chunk

        def transpose_chunk(b, ch):
            """PE-transpose the chunk into xfT (spatial on partitions)."""
            st = states[b]
            if st.setdefault("tp", 0) > ch:
                return
            st["tp"] = ch + 1
            if "xfT" not in st:
                st["xfT"] = xfT_pool.tile([P, KB, C], mm_dt, tag="xfT",
                                          name=f"xfT{b}")
            xfcch = st["xfc"][ch]
            xfT = st["xfT"]
            # two k-groups share one PSUM bank: 8 transposes, one copy
            for kk in range(0, KPC, 2):
                k = ch * KPC + kk
                tps = tps_pool.tile([P, 2, CB, P * TW], mm_dt, tag="tps")
                if TW == 1:
                    wv = tps[:]
                else:
                    wv = tps[:].rearrange("p u cb (n t) -> p u cb n t", t=TW)[
                        :, :, :, :, 0
                    ]
                for u in range(2):
                    for cb in range(CB):
                        nc.tensor.transpose(
                            wv[:, u, cb, :],
                            xfcch[:, cb, (kk + u) * P : (kk + u + 1) * P],
                            ident,
                        )
                dst = xfT[:, k : k + 2, :].rearrange("p u (cb n) -> p u cb n", n=P)
                if (ch * 2 + kk // 2) % 2 == 0:
                    nc.vector.tensor_copy(out=dst, in_=wv)
                else:
                    nc.scalar.copy(out=dst, in_=wv)

        def digest_chunk(b, ch, eng=None):
            cast16_chunk(b, ch, eng)
            cast8_chunk(b, ch, eng)
            transpose_chunk(b, ch)

        def energy_chunk(b, ch):
            """Accumulate this chunk's k-pairs into the energy PSUMs."""
            st = states[b]
            if st.setdefault("en", 0) > ch:
                return
            st["en"] = ch + 1
            if "eps" not in st:
                st["eps"] = [
                    eps_pool.tile([P, C], F32, tag="eps", name=f"eps{b}_{i}")
                    for i in range(CB)
                ]
            xfT = st["xfT"]
            for cb in range(CB):
                e_ps = st["eps"][cb]
                if DR:
                    for kk in range(0, KPC, 2):
                        k = ch * KPC + kk
                        nc.tensor.matmul(
                            e_ps[:],
                            xfT[:, k : k + 2, cb * P : (cb + 1) * P],
                            xfT[:, k : k + 2, :],
                            start=(k == 0),
                            stop=(k + 2 >= KB),
                            perf_mode=mybir.MatmulPerfMode.DoubleRow,
                        )
                else:
                    for kk in range(KPC):
                        k = ch * KPC + kk
                        nc.tensor.matmul(
                            e_ps[:],
                            xfT[:, k, cb * P : (cb + 1) * P],
                            xfT[:, k, :],
                            start=(k == 0),
                            stop=(k == KB - 1),
                        )

        def prefetch_chunk(b, ch, eng=None):
            digest_chunk(b, ch, eng)
            energy_chunk(b, ch)

        def softmax(b):
            st = states[b]
            Pmat = pmat_pool.tile([P, CB, C], mm_dt, tag="pmat")
            rS = small.tile([P, CB], F32, tag="rS")
            for cb in range(CB):
                e_ps = st["eps"][cb]
                m = small.tile([P, 1], F32, tag="m")
                nc.vector.tensor_reduce(
                    out=m[:], in_=e_ps[:], axis=mybir.AxisListType.X,
                    op=mybir.AluOpType.min,
                )
                S = small.tile([P, 1], F32, tag="S")
                nc.scalar.activation(
                    out=Pmat[:, cb, :],
                    in_=e_ps[:],
                    func=mybir.ActivationFunctionType.Exp,
                    bias=m[:],
                    scale=-1.0,
                    accum_out=S[:],
                )
                nc.vector.reciprocal(out=rS[:, cb : cb + 1], in_=S[:])

            beta = small.tile([P, CB], F32, tag="beta")
            nc.vector.tensor_tensor(
                out=beta[:],
                in0=rS[:],
                in1=gamma_sb[:].to_broadcast((P, CB)),
                op=mybir.AluOpType.mult,
            )
            st["beta"] = beta

            # PT transposes grouped by source row-block ob so each group can
            # start as soon as exp(ob) lands (no wait for all four exps).
            PT = pt_pool.tile([P, CB, C], mm_dt, tag="pt")
            for ob in range(CB):
                tps = tps_pool.tile([P, CB, P * TW], mm_dt, tag="tps")
                if TW == 1:
                    wv = tps[:]
                else:
                    wv = tps[:].rearrange("p cb (n t) -> p cb n t", t=TW)[
                        :, :, :, 0
                    ]
                for cb in range(CB):
                    nc.tensor.transpose(
                        wv[:, cb, :], Pmat[:, ob, cb * P : (cb + 1) * P], ident
                    )
                dst = PT[:, :, ob * P : (ob + 1) * P]
                if ob % 2 == 0:
                    nc.vector.tensor_copy(out=dst, in_=wv)
                else:
                    nc.scalar.copy(out=dst, in_=wv)
            st["PT"] = PT

        def mm2_chunk(b, nh):
            st = states[b]
            PT, beta = st["PT"], st["beta"]
            last = b == BPC - 1
            # epilogue split: some obs direct on DVE (scalar_tensor_tensor),
            # the rest as ACT scale-copy + gpsimd add, so no single engine
            # gates the mm2 cadence
            epi_dve = 2 if last else 3
            yv = y[b].rearrange("(ob p) n -> p ob n", p=P)
            nsl = slice(nh * NCH_SZ, (nh + 1) * NCH_SZ)
            if not last:
                ytb = ytb_pool.tile([P, CB, NCH_SZ], F32, tag="ytb")
            for ob in range(CB):
                o_ps = ops_pool.tile([P, NCH_SZ], F32, tag="ops")
                if DR:
                    for cb in range(0, CB, 2):
                        nc.tensor.matmul(
                            o_ps[:],
                            PT[:, cb : cb + 2, ob * P : (ob + 1) * P],
                            st["xfc"][nh][:, cb : cb + 2, :],
                            start=(cb == 0),
                            stop=(cb + 2 >= CB),
                            perf_mode=mybir.MatmulPerfMode.DoubleRow,
                        )
                else:
                    for cb in range(CB):
                        nc.tensor.matmul(
                            o_ps[:],
                            PT[:, cb, ob * P : (ob + 1) * P],
                            st["xfc"][nh][:, cb, :],
                            start=(cb == 0),
                            stop=(cb == CB - 1),
                        )
                if last:
                    yts = yts_pool.tile([P, NCH_SZ], F32, tag="yts")
                    tgt = yts[:]
                else:
                    tgt = ytb[:, ob, :]
                x16 = st["xf16"][nh][:, ob, :]
                if ob < epi_dve:
                    nc.vector.scalar_tensor_tensor(
                        out=tgt,
                        in0=o_ps[:],
                        scalar=beta[:, ob : ob + 1],
                        in1=x16,
                        op0=mybir.AluOpType.mult,
                        op1=mybir.AluOpType.add,
                    )
                else:
                    nc.scalar.activation(
                        out=tgt,
                        in_=o_ps[:],
                        func=mybir.ActivationFunctionType.Copy,
                        scale=beta[:, ob : ob + 1],
                    )
                    nc.gpsimd.tensor_tensor(
                        out=tgt, in0=tgt, in1=x16, op=mybir.AluOpType.add
                    )
                if last:
                    # per-(chunk, ob) writes on the by-then-idle sync HWDGE
                    # queue: each leaves right after its own epilogue
                    nc.sync.dma_start(yv[:, ob, nsl], yts[:])
            if not last:
                # SWDGE per-chunk so writes don't occupy the sync FIFO
                # ahead of the next sample's loads
                nc.gpsimd.dma_start(yv[:, :, nsl], ytb[:])

        def filler(n):
            # dependency-free dummy accumulations park the PE through the
            # softmax serial chain (row-min/exp on DVE/ACT) so HAM doesn't
            # re-throttle right before an mm2 burst
            filler_ps = ops_pool.tile([P, NCH_SZ], F32, tag="ops")
            for w in range(n):
                nc.tensor.matmul(
                    filler_ps[:], ident[:], warm_src[:],
                    start=(w == 0), stop=(w == n - 1),
                )

        # emit the first loads before everything else so the read stream
        # starts as soon as the framework preamble releases the sync queue
        load_block(0, 0)
        load_block(0, 1)

        ident = singles.tile([P, P], mm_dt)
        make_identity(nc, ident)
        gamma_sb = singles.tile([P, 1], F32)
        nc.sync.dma_start(gamma_sb[:], gamma[:].to_broadcast((P, 1)))

        # ~3.5us of dummy matmuls while the first block loads: warms the
        # PE HAM clock-gate (transpose-mode work doesn't), so the first
        # real transposes run at 2.4GHz instead of 1.2.
        warm_src = singles.tile([P, 512], mm_dt)
        nc.vector.memset(warm_src[:], 0.0)
        warm_ps = ops_pool.tile([P, NCH_SZ], F32, tag="ops", name="warm_ps")
        for w in range(16):
            nc.tensor.matmul(
                warm_ps[:], ident[:], warm_src[:],
                start=(w == 0), stop=(w == 15),
            )

        # ---- software pipeline over samples ----
        for ch in range(NCH):
            prefetch_chunk(0, ch)
        for b in range(BPC):
            nxt = b + 1
            if nxt < BPC:
                # hoist ALL of the next sample's loads: they queue right
                # behind this sample's on the sync FIFO, and the bf16/fp8
                # residency means no pool recycling ever throttles them
                for blk in range(NLD):
                    load_block(nxt, blk)
            filler(10)
            softmax(b)
            if nxt < BPC:
                # first chunk casts on gpsimd: it is idle here, and the
                # casts skip the queue of softmax work in the DVE/ACT FIFOs
                digest_chunk(nxt, 0, eng="g")
                energy_chunk(nxt, 0)
                for nh in range(NCH):
                    mm2_chunk(b, nh)
                    if nh + 1 < NCH:
                        prefetch_chunk(nxt, nh + 1)
            else:
                for nh in range(NCH):
                    mm2_chunk(b, nh)

    nc.finalize()
    return nc


def kernel(x: np.ndarray, gamma: np.ndarray) -> np.ndarray:
    global LAST_EXEC_TIME_NS, LAST_TRACE, LAST_PROFILE_JSON
    from concourse.bass_utils import run_bass_kernel_spmd

    assert x.shape == (B, C, H, W), x.shape
    x = np.ascontiguousarray(x, dtype=np.float32)
    gamma = np.ascontiguousarray(gamma, dtype=np.float32).reshape(1)

    name = MM_DT_NAME
    if name not in _CACHE:
        _CACHE[name] = _build(name)
    nc = _CACHE[name]

    xs = x.reshape(N_CORES, BPC, C, N)
    in_maps = [{"x": xs[i], "gamma": gamma} for i in range(N_CORES)]
    trace = os.environ.get("CAM_TRACE", "0") == "1"
    kwargs = {}
    if trace:
        import tempfile

        tmpdir = tempfile.mkdtemp(prefix=f"cam_trace_{name}_")
        try:
            os.unlink(f"/tmp/cam_trace_{name}")
        except OSError:
            pass
        os.symlink(tmpdir, f"/tmp/cam_trace_{name}")
        kwargs["tmpdir"] = tmpdir
    res = run_bass_kernel_spmd(
        nc, in_maps, core_ids=list(range(N_CORES)), trace=trace, **kwargs
    )
    LAST_EXEC_TIME_NS = res.exec_time_ns
    LAST_TRACE = res.instructions_and_trace
    LAST_PROFILE_JSON = res.profile_json
    out = np.concatenate([res.results[i]["y"] for i in range(N_CORES)], axis=0)
    return out.reshape(B, C, H, W)


# revision 25
# speedup vs baseline: 1.5390x; 1.5390x over previous
"""nn_CAM_Module kernel for 8 Trainium2 NeuronCores (Bass/Tile).

Contract: kernel(**inputs) takes the FULL inputs (x: [16, 512, 64, 64] fp32,
gamma: [1] fp32) and returns the FULL output, sharding batch B=16 across the
8 cores (2 samples per core, gamma replicated) — per the data-parallel
sharding: every op is a per-sample bmm, no cross-core communication.

Per-sample computation (C=512 channels, N=H*W=4096):
  energy = xf @ xf.T                          (C,C), contraction over N on PE
  m_i    = min_j energy[i,j]                  (softmax(max-e) == softmax(m-e))
  P_ij   = exp(m_i - energy_ij), S_i = sum_j  (ACT, fused row-sum)
  out    = diag(1/S) @ (P @ xf)               (PE; P^T tiles via PE transpose)
  y      = gamma * out + x                    (epilogue split DVE / ACT+gps)

v7 layout: BOTH samples' x stay resident in fp32 (8 block tiles of
[P, CB, 1024], 128KB/partition) — possible because xfT/pmat/pt run with
single buffers (their lifetimes don't overlap across samples).  Reads
stream ungated; fp32->fp8 casts are cheap (~0.42us/512 on DVE); the
epilogue reads exact fp32 x.  All loads AND writes ride the sync HWDGE
queue: one ordered stream = reads get strict priority, writes drain
behind them.  gpsimd only runs half the epilogue adds (ACT scale-copy
evacuates PSUM, gpsimd adds x), keeping DVE/ACT under the phase cadence.
"""

import os
from contextlib import ExitStack

import numpy as np

B, C, H, W = 16, 512, 64, 64
N = H * W
N_CORES = 8
BPC = B // N_CORES
P = 128

MM_DT_NAME = os.environ.get("CAM_MM_DT", "fp8")

LAST_EXEC_TIME_NS = None
LAST_TRACE = None
LAST_PROFILE_JSON = None
_CACHE = {}


def _build(mm_dt_name):
    import concourse.mybir as mybir
    import concourse.tile as tile
    from concourse import bacc
    from concourse.masks import make_identity

    F32 = mybir.dt.float32
    BF16 = mybir.dt.bfloat16
    mm_dt = {
        "bf16": mybir.dt.bfloat16,
        "fp8": mybir.dt.float8e4,
        "f32": F32,
    }[mm_dt_name]
    DR = mm_dt in (mybir.dt.float8e4, mybir.dt.float8e5)

    CB = C // P          # 4 channel blocks
    KB = N // P          # 32 spatial chunks
    NCH_SZ = 512
    NCH = N // NCH_SZ    # 8 compute chunks
    LD_SZ = 1024
    NLD = N // LD_SZ     # 4 load blocks (2 chunks each)

    nc = bacc.Bacc(None, target_bir_lowering=False, debug=False)
    x = nc.dram_tensor("x", [BPC, C, N], F32, kind="ExternalInput")
    gamma = nc.dram_tensor("gamma", [1], F32, kind="ExternalInput")
    y = nc.dram_tensor("y", [BPC, C, N], F32, kind="ExternalOutput")

    with ExitStack() as ctx:
        tc = ctx.enter_context(tile.TileContext(nc))
        singles = ctx.enter_context(tc.tile_pool(name="singles", bufs=1))
        xf_pool = ctx.enter_context(tc.tile_pool(name="xf", bufs=2 * NLD))
        xfc_pool = ctx.enter_context(tc.tile_pool(name="xfc", bufs=2 * NCH))
        xfT_pool = ctx.enter_context(tc.tile_pool(name="xfT", bufs=1))
        pmat_pool = ctx.enter_context(tc.tile_pool(name="pmat", bufs=1))
        pt_pool = ctx.enter_context(tc.tile_pool(name="pt", bufs=1))
        small = ctx.enter_context(tc.tile_pool(name="small", bufs=16))
        ytb_pool = ctx.enter_context(tc.tile_pool(name="ytb", bufs=2))
        yts_pool = ctx.enter_context(tc.tile_pool(name="yts", bufs=4))
        eps_pool = ctx.enter_context(tc.tile_pool(name="eps", bufs=4, space="PSUM"))
        tps_pool = ctx.enter_context(tc.tile_pool(name="tps", bufs=2, space="PSUM"))
        ops_pool = ctx.enter_context(tc.tile_pool(name="ops", bufs=2, space="PSUM"))

        states = {}

        def load_block(b, blk):
            """One 2MB fp32 block load on the sync HWDGE queue.  Loads and
            writes share this one ordered queue, loads emitted first, so
            the read stream has strict priority and is never starved."""
            st = states.setdefault(b, {"xf": [], "xfc": []})
            if len(st["xf"]) > blk:
                return
            xv = x[b].rearrange("(cb p) n -> p cb n", p=P)
            nsl = slice(blk * LD_SZ, (blk + 1) * LD_SZ)
            xf = xf_pool.tile([P, CB, LD_SZ], F32, tag="xf",
                              name=f"xf{b}_{blk}")
            if b == 0 and blk == 0:
                # split the very first load per-cb so the first cast8 (and
                # with it the first PE transpose) starts earlier
                for cb in range(CB):
                    nc.sync.dma_start(xf[:, cb, :], xv[:, cb, nsl])
            else:
                nc.sync.dma_start(xf[:], xv[:, :, nsl])
            st["xf"].append(xf)

        def x32_chunk(b, ch):
            blk, half = divmod(ch, 2)
            v = states[b]["xf"][blk]
            return v[:, :, half * NCH_SZ : (half + 1) * NCH_SZ]

        def copy_on(eng, out, in_):
            if eng == "v":
                nc.vector.tensor_copy(out=out, in_=in_)
            elif eng == "s":
                nc.scalar.copy(out=out, in_=in_)
            else:
                nc.gpsimd.tensor_copy(out=out, in_=in_)

        def cast8_chunk(b, ch, eng=None):
            """fp32 x -> fp8 xfc (matmul moving operand + transpose src).
            fp32-sourced casts are cheap on DVE (~0.42us/512col)."""
            st = states[b]
            if len(st["xfc"]) > ch:
                return
            load_block(b, ch // 2)
            x32 = x32_chunk(b, ch)
            xfc = xfc_pool.tile([P, CB, NCH_SZ], mm_dt, tag="xfc",
                                name=f"xfc{b}_{ch}")
            for cb in range(CB):
                copy_on(eng or "v", xfc[:, cb, :], x32[:, cb, :])
            st["xfc"].append(xfc)

        # fp8 PE-transpose writes PSUM with element step 2 (16-bit write
        # packing): stage into a 2x-strided PSUM view, copy back strided.
        TW = 2 if DR else 1
        KPC = NCH_SZ // P  # transposes-k per n-chunk

        def transpose_chunk(b, ch):
            """PE-transpose the chunk into xfT (spatial on partitions)."""
            st = states[b]
            if st.setdefault("tp", 0) > ch:
                return
            st["tp"] = ch + 1
            if "xfT" not in st:
                st["xfT"] = xfT_pool.tile([P, KB, C], mm_dt, tag="xfT",
                                          name=f"xfT{b}")
            xfcch = st["xfc"][ch]
            xfT = st["xfT"]
            # two k-groups share one PSUM bank: 8 transposes, one copy
            for kk in range(0, KPC, 2):
                k = ch * KPC + kk
                tps = tps_pool.tile([P, 2, CB, P * TW], mm_dt, tag="tps")
                if TW == 1:
                    wv = tps[:]
                else:
                    wv = tps[:].rearrange("p u cb (n t) -> p u cb n t", t=TW)[
                        :, :, :, :, 0
                    ]
                for u in range(2):
                    for cb in range(CB):
                        nc.tensor.transpose(
                            wv[:, u, cb, :],
                            xfcch[:, cb, (kk + u) * P : (kk + u + 1) * P],
                            ident,
                        )
                dst = xfT[:, k : k + 2, :].rearrange("p u (cb n) -> p u cb n", n=P)
                if (ch * 2 + kk // 2) % 4 == 0:
                    nc.vector.tensor_copy(out=dst, in_=wv)
                else:
                    nc.scalar.copy(out=dst, in_=wv)

        def digest_chunk(b, ch, eng=None):
            cast8_chunk(b, ch, eng)
            transpose_chunk(b, ch)

        def energy_chunk(b, ch):
            """Accumulate this chunk's k-pairs into the energy PSUMs."""
            st = states[b]
            if st.setdefault("en", 0) > ch:
                return
            st["en"] = ch + 1
            if "eps" not in st:
                st["eps"] = [
                    eps_pool.tile([P, C], F32, tag="eps", name=f"eps{b}_{i}")
                    for i in range(CB)
                ]
            xfT = st["xfT"]
            for cb in range(CB):
                e_ps = st["eps"][cb]
                if DR:
                    for kk in range(0, KPC, 2):
                        k = ch * KPC + kk
                        nc.tensor.matmul(
                            e_ps[:],
                            xfT[:, k : k + 2, cb * P : (cb + 1) * P],
                            xfT[:, k : k + 2, :],
                            start=(k == 0),
                            stop=(k + 2 >= KB),
                            perf_mode=mybir.MatmulPerfMode.DoubleRow,
                        )
                else:
                    for kk in range(KPC):
                        k = ch * KPC + kk
                        nc.tensor.matmul(
                            e_ps[:],
                            xfT[:, k, cb * P : (cb + 1) * P],
                            xfT[:, k, :],
                            start=(k == 0),
                            stop=(k == KB - 1),
                        )

        def prefetch_chunk(b, ch, eng=None):
            digest_chunk(b, ch, eng)
            energy_chunk(b, ch)

        def softmax(b):
            st = states[b]
            Pmat = pmat_pool.tile([P, CB, C], mm_dt, tag="pmat")
            rS = small.tile([P, CB], F32, tag="rS")
            for cb in range(CB):
                e_ps = st["eps"][cb]
                m = small.tile([P, 1], F32, tag="m")
                nc.vector.tensor_reduce(
                    out=m[:], in_=e_ps[:], axis=mybir.AxisListType.X,
                    op=mybir.AluOpType.min,
                )
                S = small.tile([P, 1], F32, tag="S")
                nc.scalar.activation(
                    out=Pmat[:, cb, :],
                    in_=e_ps[:],
                    func=mybir.ActivationFunctionType.Exp,
                    bias=m[:],
                    scale=-1.0,
                    accum_out=S[:],
                )
                nc.vector.reciprocal(out=rS[:, cb : cb + 1], in_=S[:])

            beta = small.tile([P, CB], F32, tag="beta")
            nc.vector.tensor_tensor(
                out=beta[:],
                in0=rS[:],
                in1=gamma_sb[:].to_broadcast((P, CB)),
                op=mybir.AluOpType.mult,
            )
            st["beta"] = beta

            # PT transposes grouped by source row-block ob so each group can
            # start as soon as exp(ob) lands (no wait for all four exps).
            PT = pt_pool.tile([P, CB, C], mm_dt, tag="pt")
            for ob in range(CB):
                tps = tps_pool.tile([P, CB, P * TW], mm_dt, tag="tps")
                if TW == 1:
                    wv = tps[:]
                else:
                    wv = tps[:].rearrange("p cb (n t) -> p cb n t", t=TW)[
                        :, :, :, 0
                    ]
                for cb in range(CB):
                    nc.tensor.transpose(
                        wv[:, cb, :], Pmat[:, ob, cb * P : (cb + 1) * P], ident
                    )
                dst = PT[:, :, ob * P : (ob + 1) * P]
                if ob % 2 == 0:
                    nc.vector.tensor_copy(out=dst, in_=wv)
                else:
                    nc.scalar.copy(out=dst, in_=wv)
            st["PT"] = PT

        def mm2_chunk(b, nh):
            st = states[b]
            PT, beta = st["PT"], st["beta"]
            last = b == BPC - 1
            # epilogue split: some obs direct on DVE (scalar_tensor_tensor),
            # the rest as ACT scale-copy + gpsimd add, so no single engine
            # gates the mm2 cadence
            epi_dve = 2
            yv = y[b].rearrange("(ob p) n -> p ob n", p=P)
            nsl = slice(nh * NCH_SZ, (nh + 1) * NCH_SZ)
            if not last:
                ytb = ytb_pool.tile([P, CB, NCH_SZ], F32, tag="ytb")
            for ob in range(CB):
                o_ps = ops_pool.tile([P, NCH_SZ], F32, tag="ops")
                if DR:
                    for cb in range(0, CB, 2):
                        nc.tensor.matmul(
                            o_ps[:],
                            PT[:, cb : cb + 2, ob * P : (ob + 1) * P],
                            st["xfc"][nh][:, cb : cb + 2, :],
                            start=(cb == 0),
                            stop=(cb + 2 >= CB),
                            perf_mode=mybir.MatmulPerfMode.DoubleRow,
                        )
                else:
                    for cb in range(CB):
                        nc.tensor.matmul(
                            o_ps[:],
                            PT[:, cb, ob * P : (ob + 1) * P],
                            st["xfc"][nh][:, cb, :],
                            start=(cb == 0),
                            stop=(cb == CB - 1),
                        )
                if last:
                    yts = yts_pool.tile([P, NCH_SZ], F32, tag="yts")
                    tgt = yts[:]
                else:
                    tgt = ytb[:, ob, :]
                x32 = x32_chunk(b, nh)[:, ob, :]
                if ob < epi_dve:
                    nc.vector.scalar_tensor_tensor(
                        out=tgt,
                        in0=o_ps[:],
                        scalar=beta[:, ob : ob + 1],
                        in1=x32,
                        op0=mybir.AluOpType.mult,
                        op1=mybir.AluOpType.add,
                    )
                else:
                    nc.scalar.activation(
                        out=tgt,
                        in_=o_ps[:],
                        func=mybir.ActivationFunctionType.Copy,
                        scale=beta[:, ob : ob + 1],
                    )
                    nc.gpsimd.tensor_tensor(
                        out=tgt, in0=tgt, in1=x32, op=mybir.AluOpType.add
                    )
                if last:
                    # per-(chunk, ob) writes on the by-then-idle sync HWDGE
                    # queue: each leaves right after its own epilogue
                    nc.sync.dma_start(yv[:, ob, nsl], yts[:])
            if not last:
                # per-chunk on sync HWDGE: the sync queue carries only
                # writes now (loads own the SWDGE queue), so ordering is
                # writes-after-reads by construction
                nc.sync.dma_start(yv[:, :, nsl], ytb[:])

        def filler(n):
            # dependency-free dummy accumulations park the PE through the
            # softmax serial chain (row-min/exp on DVE/ACT) so HAM doesn't
            # re-throttle right before an mm2 burst
            filler_ps = ops_pool.tile([P, NCH_SZ], F32, tag="ops")
            for w in range(n):
                nc.tensor.matmul(
                    filler_ps[:], ident[:], warm_src[:],
                    start=(w == 0), stop=(w == n - 1),
                )

        # emit the first loads before everything else so the read stream
        # starts as soon as the framework preamble releases the sync queue
        load_block(0, 0)
        load_block(0, 1)

        ident = singles.tile([P, P], mm_dt)
        make_identity(nc, ident)
        gamma_sb = singles.tile([P, 1], F32)
        nc.sync.dma_start(gamma_sb[:], gamma[:].to_broadcast((P, 1)))

        # ~3.5us of dummy matmuls while the first block loads: warms the
        # PE HAM clock-gate (transpose-mode work doesn't), so the first
        # real transposes run at 2.4GHz instead of 1.2.
        warm_src = singles.tile([P, 512], mm_dt)
        nc.vector.memset(warm_src[:], 0.0)
        warm_ps = ops_pool.tile([P, NCH_SZ], F32, tag="ops", name="warm_ps")
        for w in range(12):
            nc.tensor.matmul(
                warm_ps[:], ident[:], warm_src[:],
                start=(w == 0), stop=(w == 11),
            )

        # ---- software pipeline over samples ----
        for ch in range(NCH):
            prefetch_chunk(0, ch)
        for b in range(BPC):
            nxt = b + 1
            if nxt < BPC:
                # hoist ALL of the next sample's loads: they queue right
                # behind this sample's on the SWDGE FIFO, and the bf16/fp8
                # residency means no pool recycling ever throttles them
                for blk in range(NLD):
                    load_block(nxt, blk)
                # next sample's first chunk digest doubles as PE filler
                # through this sample's softmax serial chain (its data
                # loaded long ago; its casts only delay row-min ~1us)
                digest_chunk(nxt, 0)
            else:
                filler(8)
            softmax(b)
            if nxt < BPC:
                energy_chunk(nxt, 0)
                for nh in range(NCH):
                    mm2_chunk(b, nh)
                    if nh + 1 < NCH:
                        prefetch_chunk(nxt, nh + 1)
            else:
                for nh in range(NCH):
                    mm2_chunk(b, nh)

    nc.finalize()
    return nc


def kernel(x: np.ndarray, gamma: np.ndarray) -> np.ndarray:
    global LAST_EXEC_TIME_NS, LAST_TRACE, LAST_PROFILE_JSON
    from concourse.bass_utils import run_bass_kernel_spmd

    assert x.shape == (B, C, H, W), x.shape
    x = np.ascontiguousarray(x, dtype=np.float32)
    gamma = np.ascontiguousarray(gamma, dtype=np.float32).reshape(1)

    name = MM_DT_NAME
    if name not in _CACHE:
        _CACHE[name] = _build(name)
    nc = _CACHE[name]

    xs = x.reshape(N_CORES, BPC, C, N)
    in_maps = [{"x": xs[i], "gamma": gamma} for i in range(N_CORES)]
    trace = os.environ.get("CAM_TRACE", "0") == "1"
    kwargs = {}
    if trace:
        import tempfile

        tmpdir = tempfile.mkdtemp(prefix=f"cam_trace_{name}_")
        try:
            os.unlink(f"/tmp/cam_trace_{name}")
        except OSError:
            pass
        os.symlink(tmpdir, f"/tmp/cam_trace_{name}")
        kwargs["tmpdir"] = tmpdir
    res = run_bass_kernel_spmd(
        nc, in_maps, core_ids=list(range(N_CORES)), trace=trace, **kwargs
    )
    LAST_EXEC_TIME_NS = res.exec_time_ns
    LAST_TRACE = res.instructions_and_trace
    LAST_PROFILE_JSON = res.profile_json
    out = np.concatenate([res.results[i]["y"] for i in range(N_CORES)], axis=0)
    return out.reshape(B, C, H, W)


# revision 26
# speedup vs baseline: 1.7128x; 1.1130x over previous
"""nn_CAM_Module kernel for 8 Trainium2 NeuronCores (Bass/Tile).

Contract: kernel(**inputs) takes the FULL inputs (x: [16, 512, 64, 64] fp32,
gamma: [1] fp32) and returns the FULL output, sharding batch B=16 across the
8 cores (2 samples per core, gamma replicated) — per the data-parallel
sharding: every op is a per-sample bmm, no cross-core communication.

Per-sample computation (C=512 channels, N=H*W=4096):
  energy = xf @ xf.T                          (C,C), contraction over N on PE
  m_i    = min_j energy[i,j]                  (softmax(max-e) == softmax(m-e))
  P_ij   = exp(m_i - energy_ij), S_i = sum_j  (ACT, fused row-sum)
  out    = diag(1/S) @ (P @ xf)               (PE; P^T tiles via PE transpose)
  y      = gamma * out + x                    (epilogue split DVE / ACT+gps)

v7 layout: BOTH samples' x stay resident in fp32 (8 block tiles of
[P, CB, 1024], 128KB/partition) — possible because xfT/pmat/pt run with
single buffers (their lifetimes don't overlap across samples).  Reads
stream ungated; fp32->fp8 casts are cheap (~0.42us/512 on DVE); the
epilogue reads exact fp32 x.  All loads AND writes ride the sync HWDGE
queue: one ordered stream = reads get strict priority, writes drain
behind them.  gpsimd only runs half the epilogue adds (ACT scale-copy
evacuates PSUM, gpsimd adds x), keeping DVE/ACT under the phase cadence.
"""

import os
from contextlib import ExitStack

import numpy as np

B, C, H, W = 16, 512, 64, 64
N = H * W
N_CORES = 8
BPC = B // N_CORES
P = 128

MM_DT_NAME = os.environ.get("CAM_MM_DT", "fp8")

LAST_EXEC_TIME_NS = None
LAST_TRACE = None
LAST_PROFILE_JSON = None
_CACHE = {}


def _build(mm_dt_name):
    import concourse.mybir as mybir
    import concourse.tile as tile
    from concourse import bacc
    from concourse.masks import make_identity

    F32 = mybir.dt.float32
    BF16 = mybir.dt.bfloat16
    mm_dt = {
        "bf16": mybir.dt.bfloat16,
        "fp8": mybir.dt.float8e4,
        "f32": F32,
    }[mm_dt_name]
    DR = mm_dt in (mybir.dt.float8e4, mybir.dt.float8e5)

    CB = C // P          # 4 channel blocks
    KB = N // P          # 32 spatial chunks
    NCH_SZ = 512
    NCH = N // NCH_SZ    # 8 compute chunks
    LD_SZ = 1024
    NLD = N // LD_SZ     # 4 load blocks (2 chunks each)

    nc = bacc.Bacc(None, target_bir_lowering=False, debug=False)
    x = nc.dram_tensor("x", [BPC, C, N], F32, kind="ExternalInput")
    gamma = nc.dram_tensor("gamma", [1], F32, kind="ExternalInput")
    y = nc.dram_tensor("y", [BPC, C, N], F32, kind="ExternalOutput")

    with ExitStack() as ctx:
        tc = ctx.enter_context(tile.TileContext(nc))
        singles = ctx.enter_context(tc.tile_pool(name="singles", bufs=1))
        xf_pool = ctx.enter_context(tc.tile_pool(name="xf", bufs=2 * NLD))
        xfc_pool = ctx.enter_context(tc.tile_pool(name="xfc", bufs=2 * NCH))
        xfT_pool = ctx.enter_context(tc.tile_pool(name="xfT", bufs=1))
        pmat_pool = ctx.enter_context(tc.tile_pool(name="pmat", bufs=1))
        pt_pool = ctx.enter_context(tc.tile_pool(name="pt", bufs=1))
        small = ctx.enter_context(tc.tile_pool(name="small", bufs=16))
        ytb_pool = ctx.enter_context(tc.tile_pool(name="ytb", bufs=2))
        yts_pool = ctx.enter_context(tc.tile_pool(name="yts", bufs=4))
        eps_pool = ctx.enter_context(tc.tile_pool(name="eps", bufs=4, space="PSUM"))
        tps_pool = ctx.enter_context(tc.tile_pool(name="tps", bufs=2, space="PSUM"))
        ops_pool = ctx.enter_context(tc.tile_pool(name="ops", bufs=2, space="PSUM"))

        states = {}

        def load_block(b, blk):
            """One 2MB fp32 block load on the sync HWDGE queue.  Loads and
            writes share this one ordered queue, loads emitted first, so
            the read stream has strict priority and is never starved."""
            st = states.setdefault(b, {"xf": [], "xfc": []})
            if len(st["xf"]) > blk:
                return
            xv = x[b].rearrange("(cb p) n -> p cb n", p=P)
            nsl = slice(blk * LD_SZ, (blk + 1) * LD_SZ)
            xf = xf_pool.tile([P, CB, LD_SZ], F32, tag="xf",
                              name=f"xf{b}_{blk}")
            if b == 0 and blk == 0:
                # split the very first load per-cb so the first cast8 (and
                # with it the first PE transpose) starts earlier
                for cb in range(CB):
                    nc.sync.dma_start(xf[:, cb, :], xv[:, cb, nsl])
            else:
                nc.sync.dma_start(xf[:], xv[:, :, nsl])
            st["xf"].append(xf)

        def x32_chunk(b, ch):
            blk, half = divmod(ch, 2)
            v = states[b]["xf"][blk]
            return v[:, :, half * NCH_SZ : (half + 1) * NCH_SZ]

        def copy_on(eng, out, in_):
            if eng == "v":
                nc.vector.tensor_copy(out=out, in_=in_)
            elif eng == "s":
                nc.scalar.copy(out=out, in_=in_)
            else:
                nc.gpsimd.tensor_copy(out=out, in_=in_)

        def cast8_chunk(b, ch, eng=None):
            """fp32 x -> fp8 xfc (matmul moving operand + transpose src).
            fp32-sourced casts are cheap on DVE (~0.42us/512col)."""
            st = states[b]
            if len(st["xfc"]) > ch:
                return
            load_block(b, ch // 2)
            x32 = x32_chunk(b, ch)
            xfc = xfc_pool.tile([P, CB, NCH_SZ], mm_dt, tag="xfc",
                                name=f"xfc{b}_{ch}")
            for cb in range(CB):
                copy_on(eng or ("v" if cb % 2 == 0 else "s"),
                        xfc[:, cb, :], x32[:, cb, :])
            st["xfc"].append(xfc)

        # fp8 PE-transpose writes PSUM with element step 2 (16-bit write
        # packing): stage into a 2x-strided PSUM view, copy back strided.
        TW = 2 if DR else 1
        KPC = NCH_SZ // P  # transposes-k per n-chunk

        def transpose_chunk(b, ch):
            """PE-transpose the chunk into xfT (spatial on partitions)."""
            st = states[b]
            if st.setdefault("tp", 0) > ch:
                return
            st["tp"] = ch + 1
            if "xfT" not in st:
                st["xfT"] = xfT_pool.tile([P, KB, C], mm_dt, tag="xfT",
                                          name=f"xfT{b}")
            xfcch = st["xfc"][ch]
            xfT = st["xfT"]
            # two k-groups share one PSUM bank: 8 transposes, one copy
            for kk in range(0, KPC, 2):
                k = ch * KPC + kk
                tps = tps_pool.tile([P, 2, CB, P * TW], mm_dt, tag="tps")
                if TW == 1:
                    wv = tps[:]
                else:
                    wv = tps[:].rearrange("p u cb (n t) -> p u cb n t", t=TW)[
                        :, :, :, :, 0
                    ]
                for u in range(2):
                    for cb in range(CB):
                        nc.tensor.transpose(
                            wv[:, u, cb, :],
                            xfcch[:, cb, (kk + u) * P : (kk + u + 1) * P],
                            ident,
                        )
                dst = xfT[:, k : k + 2, :].rearrange("p u (cb n) -> p u cb n", n=P)
                if (ch * 2 + kk // 2) % 4 == 0:
                    nc.vector.tensor_copy(out=dst, in_=wv)
                else:
                    nc.scalar.copy(out=dst, in_=wv)

        def digest_chunk(b, ch, eng=None):
            cast8_chunk(b, ch, eng)
            transpose_chunk(b, ch)

        def energy_chunk(b, ch):
            """Accumulate this chunk's k-pairs into the energy PSUMs."""
            st = states[b]
            if st.setdefault("en", 0) > ch:
                return
            st["en"] = ch + 1
            if "eps" not in st:
                st["eps"] = [
                    eps_pool.tile([P, C], F32, tag="eps", name=f"eps{b}_{i}")
                    for i in range(CB)
                ]
            xfT = st["xfT"]
            for cb in range(CB):
                e_ps = st["eps"][cb]
                if DR:
                    for kk in range(0, KPC, 2):
                        k = ch * KPC + kk
                        nc.tensor.matmul(
                            e_ps[:],
                            xfT[:, k : k + 2, cb * P : (cb + 1) * P],
                            xfT[:, k : k + 2, :],
                            start=(k == 0),
                            stop=(k + 2 >= KB),
                            perf_mode=mybir.MatmulPerfMode.DoubleRow,
                        )
                else:
                    for kk in range(KPC):
                        k = ch * KPC + kk
                        nc.tensor.matmul(
                            e_ps[:],
                            xfT[:, k, cb * P : (cb + 1) * P],
                            xfT[:, k, :],
                            start=(k == 0),
                            stop=(k == KB - 1),
                        )

        def prefetch_chunk(b, ch, eng=None):
            digest_chunk(b, ch, eng)
            energy_chunk(b, ch)

        def softmax(b):
            st = states[b]
            Pmat = pmat_pool.tile([P, CB, C], mm_dt, tag="pmat")
            rS = small.tile([P, CB], F32, tag="rS")
            for cb in range(CB):
                e_ps = st["eps"][cb]
                m = small.tile([P, 1], F32, tag="m")
                nc.vector.tensor_reduce(
                    out=m[:], in_=e_ps[:], axis=mybir.AxisListType.X,
                    op=mybir.AluOpType.min,
                )
                S = small.tile([P, 1], F32, tag="S")
                nc.scalar.activation(
                    out=Pmat[:, cb, :],
                    in_=e_ps[:],
                    func=mybir.ActivationFunctionType.Exp,
                    bias=m[:],
                    scale=-1.0,
                    accum_out=S[:],
                )
                nc.vector.reciprocal(out=rS[:, cb : cb + 1], in_=S[:])

            beta = small.tile([P, CB], F32, tag="beta")
            nc.vector.tensor_tensor(
                out=beta[:],
                in0=rS[:],
                in1=gamma_sb[:].to_broadcast((P, CB)),
                op=mybir.AluOpType.mult,
            )
            st["beta"] = beta

            # PT transposes grouped by source row-block ob so each group can
            # start as soon as exp(ob) lands (no wait for all four exps).
            PT = pt_pool.tile([P, CB, C], mm_dt, tag="pt")
            for ob in range(CB):
                tps = tps_pool.tile([P, CB, P * TW], mm_dt, tag="tps")
                if TW == 1:
                    wv = tps[:]
                else:
                    wv = tps[:].rearrange("p cb (n t) -> p cb n t", t=TW)[
                        :, :, :, 0
                    ]
                for cb in range(CB):
                    nc.tensor.transpose(
                        wv[:, cb, :], Pmat[:, ob, cb * P : (cb + 1) * P], ident
                    )
                dst = PT[:, :, ob * P : (ob + 1) * P]
                if ob % 2 == 0:
                    nc.vector.tensor_copy(out=dst, in_=wv)
                else:
                    nc.scalar.copy(out=dst, in_=wv)
            st["PT"] = PT

        def mm2_chunk(b, nh):
            st = states[b]
            PT, beta = st["PT"], st["beta"]
            last = b == BPC - 1
            # epilogue split: some obs direct on DVE (scalar_tensor_tensor),
            # the rest as ACT scale-copy + gpsimd add, so no single engine
            # gates the mm2 cadence.  The last sample runs everything on
            # DVE (idle in the tail) with a 4-deep PSUM ring recycled from
            # the energy banks, so PSUM evacuation never stalls the PE.
            epi_dve = CB if last else 2
            yv = y[b].rearrange("(ob p) n -> p ob n", p=P)
            nsl = slice(nh * NCH_SZ, (nh + 1) * NCH_SZ)
            if not last:
                ytb = ytb_pool.tile([P, CB, NCH_SZ], F32, tag="ytb")
            for ob in range(CB):
                if last:
                    o_ps = eps_pool.tile([P, NCH_SZ], F32, tag="eps")
                else:
                    o_ps = ops_pool.tile([P, NCH_SZ], F32, tag="ops")
                if DR:
                    for cb in range(0, CB, 2):
                        nc.tensor.matmul(
                            o_ps[:],
                            PT[:, cb : cb + 2, ob * P : (ob + 1) * P],
                            st["xfc"][nh][:, cb : cb + 2, :],
                            start=(cb == 0),
                            stop=(cb + 2 >= CB),
                            perf_mode=mybir.MatmulPerfMode.DoubleRow,
                        )
                else:
                    for cb in range(CB):
                        nc.tensor.matmul(
                            o_ps[:],
                            PT[:, cb, ob * P : (ob + 1) * P],
                            st["xfc"][nh][:, cb, :],
                            start=(cb == 0),
                            stop=(cb == CB - 1),
                        )
                if last:
                    yts = yts_pool.tile([P, NCH_SZ], F32, tag="yts")
                    tgt = yts[:]
                else:
                    tgt = ytb[:, ob, :]
                x32 = x32_chunk(b, nh)[:, ob, :]
                if ob < epi_dve:
                    nc.vector.scalar_tensor_tensor(
                        out=tgt,
                        in0=o_ps[:],
                        scalar=beta[:, ob : ob + 1],
                        in1=x32,
                        op0=mybir.AluOpType.mult,
                        op1=mybir.AluOpType.add,
                    )
                else:
                    nc.scalar.activation(
                        out=tgt,
                        in_=o_ps[:],
                        func=mybir.ActivationFunctionType.Copy,
                        scale=beta[:, ob : ob + 1],
                    )
                    nc.gpsimd.tensor_tensor(
                        out=tgt, in0=tgt, in1=x32, op=mybir.AluOpType.add
                    )
                if last:
                    # per-(chunk, ob) writes on the by-then-idle sync HWDGE
                    # queue: each leaves right after its own epilogue
                    nc.sync.dma_start(yv[:, ob, nsl], yts[:])
            if not last:
                # per-chunk on sync HWDGE: the sync queue carries only
                # writes now (loads own the SWDGE queue), so ordering is
                # writes-after-reads by construction
                nc.sync.dma_start(yv[:, :, nsl], ytb[:])

        def filler(n):
            # dependency-free dummy accumulations park the PE through the
            # softmax serial chain (row-min/exp on DVE/ACT) so HAM doesn't
            # re-throttle right before an mm2 burst
            filler_ps = ops_pool.tile([P, NCH_SZ], F32, tag="ops")
            for w in range(n):
                nc.tensor.matmul(
                    filler_ps[:], ident[:], warm_src[:],
                    start=(w == 0), stop=(w == n - 1),
                )

        # emit the first loads before everything else so the read stream
        # starts as soon as the framework preamble releases the sync queue
        load_block(0, 0)
        load_block(0, 1)

        ident = singles.tile([P, P], mm_dt)
        make_identity(nc, ident)
        gamma_sb = singles.tile([P, 1], F32)
        nc.sync.dma_start(gamma_sb[:], gamma[:].to_broadcast((P, 1)))

        # ~3.5us of dummy matmuls while the first block loads: warms the
        # PE HAM clock-gate (transpose-mode work doesn't), so the first
        # real transposes run at 2.4GHz instead of 1.2.
        warm_src = singles.tile([P, 512], mm_dt)
        nc.vector.memset(warm_src[:], 0.0)
        warm_ps = ops_pool.tile([P, NCH_SZ], F32, tag="ops", name="warm_ps")
        for w in range(12):
            nc.tensor.matmul(
                warm_ps[:], ident[:], warm_src[:],
                start=(w == 0), stop=(w == 11),
            )

        # ---- software pipeline over samples ----
        for ch in range(NCH):
            prefetch_chunk(0, ch)
        for b in range(BPC):
            nxt = b + 1
            if nxt < BPC:
                # hoist ALL of the next sample's loads: they queue right
                # behind this sample's on the SWDGE FIFO, and the bf16/fp8
                # residency means no pool recycling ever throttles them
                for blk in range(NLD):
                    load_block(nxt, blk)
                # next sample's first chunk digest doubles as PE filler
                # through this sample's softmax serial chain (its data
                # loaded long ago; its casts only delay row-min ~1us)
                digest_chunk(nxt, 0)
            else:
                filler(8)
            softmax(b)
            if nxt < BPC:
                energy_chunk(nxt, 0)
                for nh in range(NCH):
                    # next sample's digest/energy first: its transposes and
                    # energy MMs feed the tail-critical path, and its cast8s
                    # land in the DVE/ACT FIFOs ahead of this chunk's
                    # epilogues (which wait on matmuls anyway)
                    if nh + 1 < NCH:
                        prefetch_chunk(nxt, nh + 1)
                    mm2_chunk(b, nh)
            else:
                for nh in range(NCH):
                    mm2_chunk(b, nh)

    nc.finalize()
    return nc


def kernel(x: np.ndarray, gamma: np.ndarray) -> np.ndarray:
    global LAST_EXEC_TIME_NS, LAST_TRACE, LAST_PROFILE_JSON
    from concourse.bass_utils import run_bass_kernel_spmd

    assert x.shape == (B, C, H, W), x.shape
    x = np.ascontiguousarray(x, dtype=np.float32)
    gamma = np.ascontiguousarray(gamma, dtype=np.float32).reshape(1)

    name = MM_DT_NAME
    if name not in _CACHE:
        _CACHE[name] = _build(name)
    nc = _CACHE[name]

    xs = x.reshape(N_CORES, BPC, C, N)
    in_maps = [{"x": xs[i], "gamma": gamma} for i in range(N_CORES)]
    trace = os.environ.get("CAM_TRACE", "0") == "1"
    kwargs = {}
    if trace:
        import tempfile

        tmpdir = tempfile.mkdtemp(prefix=f"cam_trace_{name}_")
        try:
            os.unlink(f"/tmp/cam_trace_{name}")
        except OSError:
            pass
        os.symlink(tmpdir, f"/tmp/cam_trace_{name}")
        kwargs["tmpdir"] = tmpdir
    res = run_bass_kernel_spmd(
        nc, in_maps, core_ids=list(range(N_CORES)), trace=trace, **kwargs
    )
    LAST_EXEC_TIME_NS = res.exec_time_ns
    LAST_TRACE = res.instructions_and_trace
    LAST_PROFILE_JSON = res.profile_json
    out = np.concatenate([res.results[i]["y"] for i in range(N_CORES)], axis=0)
    return out.reshape(B, C, H, W)
